# revision 30
# baseline (speedup 1.0000x reference)
"""Trainium2 Bass kernel for GQA causal self-attention with RoPE.

Problem: B=2, T=2048, C=1024, 16 q heads, 4 kv heads, hd=64, fp32.
Sharding: 8 cores = (batch b in {0,1}) x (kv-group g in {0..3}).
Each core computes its group's q/k/v projections (tensor parallel on the
head dim), RoPE, causal attention for its 4 q heads against its kv head,
and a partial output projection. The host sums the 4 partial outputs per
batch (the post-c_proj all-reduce done host-side) and reassembles the k/v
caches.

Device layouts are transposed ([feature, T]) so every matmul contracts
over partitions. All matmul operands are fp16 (1 cycle/row on the PE vs 2
for fp32/f32r, half the weight-load time, fp32 PSUM accumulate; ~2.4e-4
relative rounding). Softmax uses no max-subtraction (scores are bounded
for this problem's scale), so attention streams with no flash rescaling:
a ones column appended to V makes the PV matmul emit the softmax
denominator for free.
"""
import numpy as np

B, T, C = 2, 2048, 1024
N_HEAD, N_KV = 16, 4
HD = C // N_HEAD          # 64
GQ = C // N_KV            # 256 q-dims per kv group
ROPE_BASE = 10000.0
F32 = np.float32
F16 = np.float16

_CACHE = {}


def _host_tables():
    inv_freq = 1.0 / (ROPE_BASE ** (np.arange(0, HD, 2, dtype=np.float64) / HD))
    t = np.arange(T, dtype=np.float64)
    freqs = t[:, None] * inv_freq[None, :]
    emb = np.concatenate([freqs, freqs], axis=-1)      # (T, 64)
    cosT = np.cos(emb).T
    sinT = np.sin(emb).T
    P64 = np.zeros((64, 64), dtype=F16)
    for d in range(32):
        P64[d, d + 32] = -1.0
        P64[d + 32, d] = 1.0
    permT = np.zeros((128, 128), dtype=F16)            # lhsT so that rot = P64 @ q
    permT[0:64, 0:64] = P64.T
    permT[64:128, 64:128] = P64.T
    cos128 = np.concatenate([cosT, cosT], axis=0).astype(F16)   # (128, T)
    sin128 = np.concatenate([sinT, sinT], axis=0).astype(F16)
    kk, qq = np.meshgrid(np.arange(128), np.arange(128), indexing="ij")
    trimask = np.where(qq < kk, F16(-60000.0), F16(0.0))  # additive causal mask
    ident = np.eye(128, dtype=F16)
    onescol = np.ones((128, 16), dtype=F16)
    return cos128, sin128, permT, trimask, ident, onescol


def _build_bass():
    import concourse.bacc as bacc
    import concourse.mybir as mybir
    import concourse.tile as tile

    dt = mybir.dt
    f32 = dt.float32
    f16 = dt.float16
    AF = mybir.ActivationFunctionType
    ALU = mybir.AluOpType

    nc = bacc.Bacc()

    # --- dram I/O ---
    xT_d = nc.dram_tensor("xT", [C, T], f16, kind="ExternalInput")
    wq_d = nc.dram_tensor("wq_g", [C, GQ], f16, kind="ExternalInput")
    wk_d = nc.dram_tensor("wk_g", [C, HD], f16, kind="ExternalInput")
    wv_d = nc.dram_tensor("wv_g", [C, HD], f16, kind="ExternalInput")
    wo_d = nc.dram_tensor("wo_g", [GQ, C], f16, kind="ExternalInput")
    cos_d = nc.dram_tensor("cos128", [128, T], f16, kind="ExternalInput")
    sin_d = nc.dram_tensor("sin128", [128, T], f16, kind="ExternalInput")
    perm_d = nc.dram_tensor("permT", [128, 128], f16, kind="ExternalInput")
    tri_d = nc.dram_tensor("trimask", [128, 128], f16, kind="ExternalInput")
    id_d = nc.dram_tensor("ident", [128, 128], f16, kind="ExternalInput")
    oc_d = nc.dram_tensor("onescol", [128, 16], f16, kind="ExternalInput")

    po_d = nc.dram_tensor("po", [T, C], f16, kind="ExternalOutput")
    ko_d = nc.dram_tensor("ko", [HD, T], f16, kind="ExternalOutput")
    vo_d = nc.dram_tensor("vo", [HD, T], f16, kind="ExternalOutput")

    KT = C // 128     # 8 contraction tiles over C
    NT = T // 512     # 4 column chunks over T
    QT = T // 128     # 16 k/q row tiles
    scale = 1.0 / float(np.sqrt(HD))

    with tile.TileContext(nc) as tc:
        with (
            tc.tile_pool(name="const", bufs=1) as constp,
            tc.tile_pool(name="big", bufs=1) as bigp,
        ):
            # --- persistent sbuf tensors ---
            wo = bigp.tile([128, 2, C], f16)
            cos = constp.tile([128, T], f16)
            sin = constp.tile([128, T], f16)
            perm = constp.tile([128, 128], f16)
            tri = constp.tile([128, 128], f16)
            ident = constp.tile([128, 128], f16)
            qT = bigp.tile([128, 2, T], f16)    # rope'd Q^T, head pairs (0,1),(2,3)
            kT = bigp.tile([64, T], f16)        # rope'd K^T
            kT2 = bigp.tile([128, T], f16)      # rope'd K^T copy at partitions 64:127
            vT = bigp.tile([64, T], f16)
            vaug = bigp.tile([128, QT, 65], f16)  # V rows + ones col
            attnT = bigp.tile([128, 2, T], f16)   # normalized attn out, transposed

            # ---------------- phase 1+2: projections + RoPE ----------------
            with (
                tc.tile_pool(name="xw", bufs=1) as xwp,
                tc.tile_pool(name="ppool", bufs=4, space="PSUM") as ppool,
                tc.tile_pool(name="rpool", bufs=2, space="PSUM") as rpool,
                tc.tile_pool(name="tmp", bufs=2) as tmpp,
            ):
                xT = xwp.tile([128, KT, T], f16)
                wq = xwp.tile([128, KT, GQ], f16)
                wk = xwp.tile([128, KT, HD], f16)
                wv = xwp.tile([128, KT, HD], f16)
                nc.sync.dma_start(perm[:], perm_d[:])
                nc.sync.dma_start(wk[:], wk_d[:].rearrange("(k p) n -> p k n", p=128))
                nc.sync.dma_start(wq[:], wq_d[:].rearrange("(k p) n -> p k n", p=128))
                xT_r = xT_d[:].rearrange("(k p) t -> p k t", p=128)
                for k in range(KT):
                    nc.sync.dma_start(xT[:, k, :], xT_r[:, k, :])
                nc.sync.dma_start(wv[:], wv_d[:].rearrange("(k p) n -> p k n", p=128))
                nc.sync.dma_start(cos[:], cos_d[:])
                nc.sync.dma_start(sin[:], sin_d[:])
                nc.sync.dma_start(tri[:], tri_d[:])
                nc.sync.dma_start(ident[:], id_d[:])
                nc.sync.dma_start(vaug[:, :, 64], oc_d[:])
                nc.sync.dma_start(wo[:], wo_d[:].rearrange("(k p) n -> p k n", p=128))

                # ~5us of dummy matmuls on the first-arrived const warms
                # the PE HAM clock gate before the real work lands.
                wps = ppool.tile([128, 512], f32, tag="proj", name="wps")
                for _ in range(72):
                    nc.tensor.matmul(wps[:, 0:128], perm[:], perm[:],
                                     start=True, stop=True)

                def proj(w_sb, m, out_cb):
                    """One [rows, T] projection; lhsT reused across the 4
                    n-chunks of each k-tile so weight loads amortize."""
                    rows = 128 if m >= 0 else HD
                    pss = [ppool.tile([128, 512], f32, tag="proj", name=f"psp{n}")
                           for n in range(NT)]
                    for k in range(KT):
                        lhsT = (w_sb[:, k, m * 128:(m + 1) * 128] if m >= 0
                                else w_sb[:, k, :])
                        for n in range(NT):
                            nc.tensor.matmul(
                                pss[n][0:rows, :], lhsT,
                                xT[:, k, n * 512:(n + 1) * 512],
                                start=(k == 0), stop=(k == KT - 1),
                            )
                    for n in range(NT):
                        out_cb(n, pss[n][0:rows, :])

                def rope(raw_get, out_fn, rows):
                    """rot = perm @ raw (PE); out_fn applies cos/sin (DVE).
                    Processes T in 2 half-chunks of 1024."""
                    pslice = perm[:] if rows == 128 else perm[0:64, 0:64]
                    for c2 in range(2):
                        pr = rpool.tile([128, 2, 512], f32, tag="rope", name="pr")
                        for i in range(2):
                            nc.tensor.matmul(
                                pr[0:rows, i, :], pslice,
                                raw_get(c2 * 2 + i), start=True, stop=True,
                            )
                        h_sl = slice(c2 * 1024, (c2 + 1) * 1024)
                        rt = tmpp.tile([128, 1024], f16, tag="ropetmp", name="rt")
                        nc.vector.tensor_mul(
                            rt[0:rows, :],
                            pr[0:rows, :, :].rearrange("p a b -> p (a b)"),
                            sin[0:rows, h_sl])
                        out_fn(h_sl, rt)

                # K proj -> kT (raw), rope in place, store k cache
                proj(wk, -1, lambda n, ps: nc.vector.tensor_copy(
                    kT[:, n * 512:(n + 1) * 512], ps))

                def k_rope_out(h_sl, rt):
                    nc.vector.tensor_mul(kT[:, h_sl], kT[:, h_sl], cos[0:64, h_sl])
                    nc.vector.tensor_add(kT[:, h_sl], kT[:, h_sl], rt[0:64, :])
                rope(lambda n: kT[:, n * 512:(n + 1) * 512], k_rope_out, HD)
                nc.sync.dma_start(ko_d[:], kT[:])
                nc.vector.tensor_copy(kT2[64:128, :], kT[:])

                # Q proj (2 head-pair tiles) -> qT raw, rope into qT/qodd
                for m in range(2):
                    proj(wq, m, lambda n, ps, m=m: nc.vector.tensor_copy(
                        qT[:, m, n * 512:(n + 1) * 512], ps))

                    def q_rope_out(h_sl, rt, m=m):
                        nc.vector.tensor_mul(
                            qT[:, m, h_sl], qT[:, m, h_sl], cos[:, h_sl])
                        nc.vector.tensor_add(
                            qT[:, m, h_sl], qT[:, m, h_sl], rt[:])
                    rope(lambda n, m=m: qT[:, m, n * 512:(n + 1) * 512],
                         q_rope_out, 128)

                # V proj -> vT, then transpose into vaug [T, 64] (+ ones col)
                proj(wv, -1, lambda n, ps: nc.vector.tensor_copy(
                    vT[:, n * 512:(n + 1) * 512], ps))
                nc.sync.dma_start(vo_d[:], vT[:])
                for t in range(QT):
                    pv = ppool.tile([128, 64], f16, tag="proj", name="pv")
                    nc.tensor.transpose(
                        pv[:, 0:64], vT[:, t * 128:(t + 1) * 128], ident[0:64, 0:64])
                    nc.vector.tensor_copy(vaug[:, t, 0:64], pv[:, 0:64])
                # bridge the proj->attention transition so the PE clock gate
                # stays lifted across the phase hand-off
                wpe = ppool.tile([128, 512], f32, tag="proj", name="wpe")
                for _ in range(30):
                    nc.tensor.matmul(wpe[:, 0:128], perm[:], perm[:],
                                     start=True, stop=True)

            # ---------------- phase 3: attention ----------------
            with (
                tc.tile_pool(name="spool", bufs=2, space="PSUM") as spool,
                tc.tile_pool(name="opool", bufs=1, space="PSUM") as opool,
                tc.tile_pool(name="ptpool", bufs=4) as ptpool,
                tc.tile_pool(name="npool", bufs=2) as npool,
            ):
                for qc in range(NT):
                    po = opool.tile([65, 4, 512], f32, tag="pvo", name="po")
                    # dense burst of junk matmuls: covers a full HAM activity
                    # window so the PE clock-gate lifts to 8/8 for the chunk.
                    # Results land in po rows 0:64 and are overwritten by the
                    # first PV accumulation (start=True).
                    for _ in range(36):
                        nc.tensor.matmul(po[0:64, 0, 0:128], perm[:, 0:64],
                                         perm[:, 0:128], start=True, stop=True)
                    njt = 4 * qc + 4            # k tiles 0..njt-1
                    # software-pipelined: emit each unit's PV one unit
                    # late so the in-order PE queue never stalls on the exp
                    # the PV consumes — the next unit's S runs instead.
                    pending = []

                    def flush_pv():
                        for (pj, pm, pT_, pqlo) in pending:
                            for hb in range(2):
                                nc.tensor.matmul(
                                    po[:, 2 * pm + hb, pqlo:512],
                                    vaug[:, pj, :], pT_[:, hb, pqlo:512],
                                    start=(pj == 0), stop=(pj == njt - 1),
                                    skip_group_check=True,
                                )
                        pending.clear()

                    for j in range(njt):
                        dd = j - 4 * qc
                        qlo = max(0, dd * 128)
                        kslice = kT[:, j * 128:(j + 1) * 128]
                        for m in range(2):      # head pair (2m, 2m+1)
                            ps = spool.tile([128, 2, 512], f32, tag="st", name="ps_s")
                            for hb in range(2):
                                qh = qT[hb * 64:(hb + 1) * 64, m, :]
                                kk_sl = (kT[:, j * 128:(j + 1) * 128] if hb == 0
                                         else kT2[64:128, j * 128:(j + 1) * 128])
                                nc.tensor.matmul(
                                    ps[:, hb, qlo:512], kk_sl,
                                    qh[:, qc * 512 + qlo:(qc + 1) * 512],
                                    start=True, stop=(dd < 0),
                                    tile_position=(hb * 64, 0),
                                )
                            if dd >= 0:
                                # causal triangle: accumulate the additive
                                # mask on the PE (ident^T @ tri = tri), after
                                # both row-tiled S matmuls have been issued
                                for hb in range(2):
                                    nc.tensor.matmul(
                                        ps[:, hb, dd * 128:(dd + 1) * 128],
                                        ident[:], tri[:],
                                        start=False, stop=True,
                                        skip_group_check=True,
                                    )
                            pT = ptpool.tile([128, 2, 512], f16, tag="pt", name="pT")
                            nc.scalar.activation(
                                pT[:, :, qlo:512], ps[:, :, qlo:512],
                                AF.Exp, scale=scale)
                            flush_pv()
                            pending.append((j, m, pT, qlo))
                    flush_pv()
                    # normalize: attnT[h] = num / l (l = po row 64). One ACT
                    # copy frees the po bank quickly so the next chunk's PV can
                    # start; the divide chain then runs off the critical path.
                    sn = npool.tile([96, 4, 512], f32, tag="sn", name="sn")
                    nc.vector.tensor_copy(sn[0:65, :, :], po[:])
                    # reciprocal of the l row: 32x32 stream-transpose so the
                    # 2048 values spread over 32 partitions (free dim 64)
                    # instead of one 2048-wide single-partition op.
                    lt = npool.tile([32, 2048], f32, tag="lt", name="lt")
                    nc.vector.transpose(
                        lt[:], sn[64:96, :, :].rearrange("p a b -> p (a b)"))
                    ltv = lt[:, 0:2048].rearrange("p (a b) -> p a b", b=32)[:, :, 0]
                    nc.vector.reciprocal(ltv, ltv)
                    rec = npool.tile([32, 2048], f32, tag="rec", name="rec")
                    nc.vector.transpose(rec[:], lt[:])
                    recb = npool.tile([64, 4, 512], f32, tag="recb", name="recb")
                    nc.gpsimd.partition_broadcast(
                        recb[:].rearrange("p a b -> p (a b)"), rec[0:1, :])
                    for h in range(4):
                        m, hb = divmod(h, 2)
                        eng = nc.vector if h % 2 == 0 else nc.gpsimd
                        eng.tensor_mul(
                            attnT[hb * 64:(hb + 1) * 64, m, qc * 512:(qc + 1) * 512],
                            sn[0:64, h, :], recb[:, h, :])

            # ---------------- phase 4: output projection ----------------
            with (
                tc.tile_pool(name="oppool", bufs=4, space="PSUM") as oppool,
                tc.tile_pool(name="ostage", bufs=4) as ostage,
            ):
                for t in range(QT):
                    pps = [oppool.tile([128, 512], f32, tag="op", name="pp")
                           for _ in range(2)]
                    for kk in range(2):
                        for n2 in range(2):
                            nc.tensor.matmul(
                                pps[n2][:],
                                attnT[:, kk, t * 128:(t + 1) * 128],
                                wo[:, kk, n2 * 512:(n2 + 1) * 512],
                                start=(kk == 0), stop=(kk == 1),
                            )
                    for n2 in range(2):
                        ot = ostage.tile([128, 512], f16, tag="ot", name="ot")
                        nc.vector.tensor_copy(ot[:], pps[n2][:])
                        nc.sync.dma_start(
                            po_d[t * 128:(t + 1) * 128, n2 * 512:(n2 + 1) * 512], ot[:])

    nc.compile()
    return nc


def _get_nc():
    if "nc" not in _CACHE:
        _CACHE["nc"] = _build_bass()
    return _CACHE["nc"]


def _make_in_maps(x, wq, wk, wv, wo):
    x = np.asarray(x, dtype=F32)
    wq = np.asarray(wq, dtype=F32)
    wk = np.asarray(wk, dtype=F32)
    wv = np.asarray(wv, dtype=F32)
    wo = np.asarray(wo, dtype=F32)
    cos128, sin128, permT, trimask, ident, onescol = _host_tables()
    in_maps = []
    for cid in range(8):
        b, g = divmod(cid, 4)
        in_maps.append({
            "xT": np.ascontiguousarray(x[b].T).astype(F16),
            "wq_g": wq[:, g * GQ:(g + 1) * GQ].astype(F16),
            "wk_g": wk[:, g * HD:(g + 1) * HD].astype(F16),
            "wv_g": wv[:, g * HD:(g + 1) * HD].astype(F16),
            "wo_g": np.ascontiguousarray(wo[g * GQ:(g + 1) * GQ, :]).astype(F16),
            "cos128": cos128, "sin128": sin128, "permT": permT,
            "trimask": trimask, "ident": ident, "onescol": onescol,
        })
    return in_maps


def _gather(results):
    out = np.zeros((B, T, C), F32)
    k_out = np.zeros((B, N_KV, T, HD), F32)
    v_out = np.zeros((B, N_KV, T, HD), F32)
    for cid in range(8):
        b, g = divmod(cid, 4)
        out[b] += results[cid]["po"].astype(F32)
        k_out[b, g] = results[cid]["ko"].T.astype(F32)
        v_out[b, g] = results[cid]["vo"].T.astype(F32)
    return out, k_out, v_out


def kernel(x, wq, wk, wv, wo):
    from concourse.bass_utils import run_bass_kernel_spmd

    in_maps = _make_in_maps(x, wq, wk, wv, wo)
    nc = _get_nc()
    res = run_bass_kernel_spmd(nc, in_maps, core_ids=list(range(8)))
    return _gather(res.results)


# revision 32
# speedup vs baseline: 1.0279x; 1.0279x over previous
"""Trainium2 Bass kernel for GQA causal self-attention with RoPE.

Problem: B=2, T=2048, C=1024, 16 q heads, 4 kv heads, hd=64, fp32.
Sharding: 8 cores = (batch b in {0,1}) x (kv-group g in {0..3}).
Each core computes its group's q/k/v projections (tensor parallel on the
head dim), RoPE, causal attention for its 4 q heads against its kv head,
and a partial output projection. The host sums the 4 partial outputs per
batch (the post-c_proj all-reduce done host-side) and reassembles the k/v
caches.

Device layouts are transposed ([feature, T]) so every matmul contracts
over partitions. All matmul operands are fp16 (1 cycle/row on the PE vs 2
for fp32/f32r, half the weight-load time, fp32 PSUM accumulate; ~2.4e-4
relative rounding). Softmax uses no max-subtraction (scores are bounded
for this problem's scale), so attention streams with no flash rescaling:
a ones column appended to V makes the PV matmul emit the softmax
denominator for free.
"""
import numpy as np

B, T, C = 2, 2048, 1024
N_HEAD, N_KV = 16, 4
HD = C // N_HEAD          # 64
GQ = C // N_KV            # 256 q-dims per kv group
ROPE_BASE = 10000.0
F32 = np.float32
F16 = np.float16

_CACHE = {}


def _host_tables():
    inv_freq = 1.0 / (ROPE_BASE ** (np.arange(0, HD, 2, dtype=np.float64) / HD))
    t = np.arange(T, dtype=np.float64)
    freqs = t[:, None] * inv_freq[None, :]
    emb = np.concatenate([freqs, freqs], axis=-1)      # (T, 64)
    cosT = np.cos(emb).T
    sinT = np.sin(emb).T
    P64 = np.zeros((64, 64), dtype=F16)
    for d in range(32):
        P64[d, d + 32] = -1.0
        P64[d + 32, d] = 1.0
    permT = np.zeros((128, 128), dtype=F16)            # lhsT so that rot = P64 @ q
    permT[0:64, 0:64] = P64.T
    permT[64:128, 64:128] = P64.T
    cos128 = np.concatenate([cosT, cosT], axis=0).astype(F16)   # (128, T)
    sin128 = np.concatenate([sinT, sinT], axis=0).astype(F16)
    kk, qq = np.meshgrid(np.arange(128), np.arange(128), indexing="ij")
    trimask = np.where(qq < kk, F16(-60000.0), F16(0.0))  # additive causal mask
    ident = np.eye(128, dtype=F16)
    onescol = np.ones((128, 16), dtype=F16)
    return cos128, sin128, permT, trimask, ident, onescol


def _build_bass():
    import concourse.bacc as bacc
    import concourse.mybir as mybir
    import concourse.tile as tile

    dt = mybir.dt
    f32 = dt.float32
    f16 = dt.float16
    AF = mybir.ActivationFunctionType
    ALU = mybir.AluOpType

    nc = bacc.Bacc()

    # --- dram I/O ---
    xT_d = nc.dram_tensor("xT", [C, T], f16, kind="ExternalInput")
    wq_d = nc.dram_tensor("wq_g", [C, GQ], f16, kind="ExternalInput")
    wk_d = nc.dram_tensor("wk_g", [C, HD], f16, kind="ExternalInput")
    wv_d = nc.dram_tensor("wv_g", [C, HD], f16, kind="ExternalInput")
    wo_d = nc.dram_tensor("wo_g", [GQ, C], f16, kind="ExternalInput")
    cos_d = nc.dram_tensor("cos128", [128, T], f16, kind="ExternalInput")
    sin_d = nc.dram_tensor("sin128", [128, T], f16, kind="ExternalInput")
    perm_d = nc.dram_tensor("permT", [128, 128], f16, kind="ExternalInput")
    tri_d = nc.dram_tensor("trimask", [128, 128], f16, kind="ExternalInput")
    id_d = nc.dram_tensor("ident", [128, 128], f16, kind="ExternalInput")
    oc_d = nc.dram_tensor("onescol", [128, 16], f16, kind="ExternalInput")

    po_d = nc.dram_tensor("po", [T, C], f16, kind="ExternalOutput")
    ko_d = nc.dram_tensor("ko", [HD, T], f16, kind="ExternalOutput")
    vo_d = nc.dram_tensor("vo", [HD, T], f16, kind="ExternalOutput")

    KT = C // 128     # 8 contraction tiles over C
    NT = T // 512     # 4 column chunks over T
    QT = T // 128     # 16 k/q row tiles
    scale = 1.0 / float(np.sqrt(HD))

    with tile.TileContext(nc) as tc:
        with (
            tc.tile_pool(name="const", bufs=1) as constp,
            tc.tile_pool(name="big", bufs=1) as bigp,
        ):
            # --- persistent sbuf tensors ---
            wo = bigp.tile([128, 2, C], f16)
            cos = constp.tile([128, T], f16)
            sin = constp.tile([128, T], f16)
            perm = constp.tile([128, 128], f16)
            tri = constp.tile([128, 128], f16)
            ident = constp.tile([128, 128], f16)
            qT = bigp.tile([128, 2, T], f16)    # rope'd Q^T, head pairs (0,1),(2,3)
            kT = bigp.tile([64, T], f16)        # rope'd K^T
            kT2 = bigp.tile([128, T], f16)      # rope'd K^T copy at partitions 64:127
            vT = bigp.tile([64, T], f16)
            vaug = bigp.tile([128, QT, 65], f16)  # V rows + ones col
            attnT = bigp.tile([128, 2, T], f16)   # normalized attn out, transposed

            # ---------------- phase 1+2: projections + RoPE ----------------
            with (
                tc.tile_pool(name="xw", bufs=1) as xwp,
                tc.tile_pool(name="ppool", bufs=4, space="PSUM") as ppool,
                tc.tile_pool(name="rpool", bufs=2, space="PSUM") as rpool,
                tc.tile_pool(name="tmp", bufs=2) as tmpp,
            ):
                xT = xwp.tile([128, KT, T], f16)
                wq = xwp.tile([128, KT, GQ], f16)
                wk = xwp.tile([128, KT, HD], f16)
                wv = xwp.tile([128, KT, HD], f16)
                nc.sync.dma_start(perm[:], perm_d[:])
                nc.sync.dma_start(wk[:], wk_d[:].rearrange("(k p) n -> p k n", p=128))
                xT_r = xT_d[:].rearrange("(k p) t -> p k t", p=128)
                for k in range(2):
                    nc.sync.dma_start(xT[:, k, :], xT_r[:, k, :])
                nc.sync.dma_start(wq[:], wq_d[:].rearrange("(k p) n -> p k n", p=128))
                for k in range(2, KT):
                    nc.sync.dma_start(xT[:, k, :], xT_r[:, k, :])
                nc.sync.dma_start(wv[:], wv_d[:].rearrange("(k p) n -> p k n", p=128))
                nc.sync.dma_start(cos[:], cos_d[:])
                nc.sync.dma_start(sin[:], sin_d[:])
                nc.sync.dma_start(tri[:], tri_d[:])
                nc.sync.dma_start(ident[:], id_d[:])
                nc.sync.dma_start(vaug[:, :, 64], oc_d[:])
                nc.sync.dma_start(wo[:], wo_d[:].rearrange("(k p) n -> p k n", p=128))

                # ~5us of dummy matmuls on the first-arrived const warms
                # the PE HAM clock gate before the real work lands.
                wps = ppool.tile([128, 512], f32, tag="proj", name="wps")
                for _ in range(48):
                    nc.tensor.matmul(wps[:, 0:128], perm[:], perm[:],
                                     start=True, stop=True)

                def proj(w_sb, m, out_cb):
                    """One [rows, T] projection; lhsT reused across the 4
                    n-chunks of each k-tile so weight loads amortize."""
                    rows = 128 if m >= 0 else HD
                    pss = [ppool.tile([128, 512], f32, tag="proj", name=f"psp{n}")
                           for n in range(NT)]
                    for k in range(KT):
                        lhsT = (w_sb[:, k, m * 128:(m + 1) * 128] if m >= 0
                                else w_sb[:, k, :])
                        for n in range(NT):
                            nc.tensor.matmul(
                                pss[n][0:rows, :], lhsT,
                                xT[:, k, n * 512:(n + 1) * 512],
                                start=(k == 0), stop=(k == KT - 1),
                            )
                    for n in range(NT):
                        out_cb(n, pss[n][0:rows, :])

                def rope(raw_get, out_fn, rows):
                    """rot = perm @ raw (PE); out_fn applies cos/sin (DVE).
                    Processes T in 2 half-chunks of 1024."""
                    pslice = perm[:] if rows == 128 else perm[0:64, 0:64]
                    for c2 in range(2):
                        pr = rpool.tile([128, 2, 512], f32, tag="rope", name="pr")
                        for i in range(2):
                            nc.tensor.matmul(
                                pr[0:rows, i, :], pslice,
                                raw_get(c2 * 2 + i), start=True, stop=True,
                            )
                        h_sl = slice(c2 * 1024, (c2 + 1) * 1024)
                        rt = tmpp.tile([128, 1024], f16, tag="ropetmp", name="rt")
                        nc.vector.tensor_mul(
                            rt[0:rows, :],
                            pr[0:rows, :, :].rearrange("p a b -> p (a b)"),
                            sin[0:rows, h_sl])
                        out_fn(h_sl, rt)

                # K proj -> kT (raw), rope in place, store k cache
                proj(wk, -1, lambda n, ps: nc.scalar.activation(
                    kT[:, n * 512:(n + 1) * 512], ps, AF.Copy))

                def k_rope_out(h_sl, rt):
                    nc.vector.tensor_mul(kT[:, h_sl], kT[:, h_sl], cos[0:64, h_sl])
                    nc.vector.tensor_add(kT[:, h_sl], kT[:, h_sl], rt[0:64, :])
                rope(lambda n: kT[:, n * 512:(n + 1) * 512], k_rope_out, HD)
                nc.sync.dma_start(ko_d[:], kT[:])
                nc.vector.tensor_copy(kT2[64:128, :], kT[:])

                # Q proj (2 head-pair tiles) -> qT raw, rope into qT/qodd
                for m in range(2):
                    proj(wq, m, lambda n, ps, m=m: nc.scalar.activation(
                        qT[:, m, n * 512:(n + 1) * 512], ps, AF.Copy))

                    def q_rope_out(h_sl, rt, m=m):
                        nc.vector.tensor_mul(
                            qT[:, m, h_sl], qT[:, m, h_sl], cos[:, h_sl])
                        nc.vector.tensor_add(
                            qT[:, m, h_sl], qT[:, m, h_sl], rt[:])
                    rope(lambda n, m=m: qT[:, m, n * 512:(n + 1) * 512],
                         q_rope_out, 128)

                # V proj -> vT, then transpose into vaug [T, 64] (+ ones col)
                proj(wv, -1, lambda n, ps: nc.scalar.activation(
                    vT[:, n * 512:(n + 1) * 512], ps, AF.Copy))
                nc.sync.dma_start(vo_d[:], vT[:])
                for t in range(QT):
                    pv = ppool.tile([128, 64], f16, tag="proj", name="pv")
                    nc.tensor.transpose(
                        pv[:, 0:64], vT[:, t * 128:(t + 1) * 128], ident[0:64, 0:64])
                    nc.vector.tensor_copy(vaug[:, t, 0:64], pv[:, 0:64])
                # bridge the proj->attention transition so the PE clock gate
                # stays lifted across the phase hand-off
                wpe = ppool.tile([128, 512], f32, tag="proj", name="wpe")
                for _ in range(30):
                    nc.tensor.matmul(wpe[:, 0:128], perm[:], perm[:],
                                     start=True, stop=True)

            # ---------------- phase 3: attention ----------------
            with (
                tc.tile_pool(name="spool", bufs=2, space="PSUM") as spool,
                tc.tile_pool(name="opool", bufs=1, space="PSUM") as opool,
                tc.tile_pool(name="ptpool", bufs=4) as ptpool,
                tc.tile_pool(name="npool", bufs=2) as npool,
            ):
                for qc in range(NT):
                    po = opool.tile([65, 4, 512], f32, tag="pvo", name="po")
                    # dense burst of junk matmuls: covers a full HAM activity
                    # window so the PE clock-gate lifts to 8/8 for the chunk.
                    # Results land in po rows 0:64 and are overwritten by the
                    # first PV accumulation (start=True).
                    for _ in range(36):
                        nc.tensor.matmul(po[0:64, 0, 0:128], perm[:, 0:64],
                                         perm[:, 0:128], start=True, stop=True)
                    njt = 4 * qc + 4            # k tiles 0..njt-1
                    # software-pipelined: emit each unit's PV one unit
                    # late so the in-order PE queue never stalls on the exp
                    # the PV consumes — the next unit's S runs instead.
                    pending = []

                    def flush_pv():
                        for (pj, pm, pT_, pqlo) in pending:
                            for hb in range(2):
                                nc.tensor.matmul(
                                    po[:, 2 * pm + hb, pqlo:512],
                                    vaug[:, pj, :], pT_[:, hb, pqlo:512],
                                    start=(pj == 0), stop=(pj == njt - 1),
                                    skip_group_check=True,
                                )
                        pending.clear()

                    for j in range(njt):
                        dd = j - 4 * qc
                        qlo = max(0, dd * 128)
                        kslice = kT[:, j * 128:(j + 1) * 128]
                        for m in range(2):      # head pair (2m, 2m+1)
                            ps = spool.tile([128, 2, 512], f32, tag="st", name="ps_s")
                            for hb in range(2):
                                qh = qT[hb * 64:(hb + 1) * 64, m, :]
                                kk_sl = (kT[:, j * 128:(j + 1) * 128] if hb == 0
                                         else kT2[64:128, j * 128:(j + 1) * 128])
                                nc.tensor.matmul(
                                    ps[:, hb, qlo:512], kk_sl,
                                    qh[:, qc * 512 + qlo:(qc + 1) * 512],
                                    start=True, stop=(dd < 0),
                                    tile_position=(hb * 64, 0),
                                )
                            if dd >= 0:
                                # causal triangle: accumulate the additive
                                # mask on the PE (ident^T @ tri = tri), after
                                # both row-tiled S matmuls have been issued
                                for hb in range(2):
                                    nc.tensor.matmul(
                                        ps[:, hb, dd * 128:(dd + 1) * 128],
                                        ident[:], tri[:],
                                        start=False, stop=True,
                                        skip_group_check=True,
                                    )
                            pT = ptpool.tile([128, 2, 512], f16, tag="pt", name="pT")
                            nc.scalar.activation(
                                pT[:, :, qlo:512], ps[:, :, qlo:512],
                                AF.Exp, scale=scale)
                            flush_pv()
                            pending.append((j, m, pT, qlo))
                    flush_pv()
                    # normalize: attnT[h] = num / l (l = po row 64). One ACT
                    # copy frees the po bank quickly so the next chunk's PV can
                    # start; the divide chain then runs off the critical path.
                    sn = npool.tile([96, 4, 512], f32, tag="sn", name="sn")
                    nc.vector.tensor_copy(sn[0:65, :, :], po[:])
                    # reciprocal of the l row: 32x32 stream-transpose so the
                    # 2048 values spread over 32 partitions (free dim 64)
                    # instead of one 2048-wide single-partition op.
                    lt = npool.tile([32, 2048], f32, tag="lt", name="lt")
                    nc.vector.transpose(
                        lt[:], sn[64:96, :, :].rearrange("p a b -> p (a b)"))
                    ltv = lt[:, 0:2048].rearrange("p (a b) -> p a b", b=32)[:, :, 0]
                    nc.vector.reciprocal(ltv, ltv)
                    rec = npool.tile([32, 2048], f32, tag="rec", name="rec")
                    nc.vector.transpose(rec[:], lt[:])
                    recb = npool.tile([64, 4, 512], f32, tag="recb", name="recb")
                    nc.gpsimd.partition_broadcast(
                        recb[:].rearrange("p a b -> p (a b)"), rec[0:1, :])
                    for h in range(4):
                        m, hb = divmod(h, 2)
                        eng = nc.vector if h % 2 == 0 else nc.gpsimd
                        eng.tensor_mul(
                            attnT[hb * 64:(hb + 1) * 64, m, qc * 512:(qc + 1) * 512],
                            sn[0:64, h, :], recb[:, h, :])

            # ---------------- phase 4: output projection ----------------
            with (
                tc.tile_pool(name="oppool", bufs=4, space="PSUM") as oppool,
                tc.tile_pool(name="ostage", bufs=4) as ostage,
            ):
                for t in range(QT):
                    pps = [oppool.tile([128, 512], f32, tag="op", name="pp")
                           for _ in range(2)]
                    for kk in range(2):
                        for n2 in range(2):
                            nc.tensor.matmul(
                                pps[n2][:],
                                attnT[:, kk, t * 128:(t + 1) * 128],
                                wo[:, kk, n2 * 512:(n2 + 1) * 512],
                                start=(kk == 0), stop=(kk == 1),
                            )
                    for n2 in range(2):
                        ot = ostage.tile([128, 512], f16, tag="ot", name="ot")
                        if (t + n2) % 2 == 0:
                            nc.vector.tensor_copy(ot[:], pps[n2][:])
                        else:
                            nc.scalar.activation(ot[:], pps[n2][:], AF.Copy)
                        nc.sync.dma_start(
                            po_d[t * 128:(t + 1) * 128, n2 * 512:(n2 + 1) * 512], ot[:])

    nc.compile()
    return nc


def _get_nc():
    if "nc" not in _CACHE:
        _CACHE["nc"] = _build_bass()
    return _CACHE["nc"]


def _make_in_maps(x, wq, wk, wv, wo):
    x = np.asarray(x, dtype=F32)
    wq = np.asarray(wq, dtype=F32)
    wk = np.asarray(wk, dtype=F32)
    wv = np.asarray(wv, dtype=F32)
    wo = np.asarray(wo, dtype=F32)
    cos128, sin128, permT, trimask, ident, onescol = _host_tables()
    in_maps = []
    for cid in range(8):
        b, g = divmod(cid, 4)
        in_maps.append({
            "xT": np.ascontiguousarray(x[b].T).astype(F16),
            "wq_g": wq[:, g * GQ:(g + 1) * GQ].astype(F16),
            "wk_g": wk[:, g * HD:(g + 1) * HD].astype(F16),
            "wv_g": wv[:, g * HD:(g + 1) * HD].astype(F16),
            "wo_g": np.ascontiguousarray(wo[g * GQ:(g + 1) * GQ, :]).astype(F16),
            "cos128": cos128, "sin128": sin128, "permT": permT,
            "trimask": trimask, "ident": ident, "onescol": onescol,
        })
    return in_maps


def _gather(results):
    out = np.zeros((B, T, C), F32)
    k_out = np.zeros((B, N_KV, T, HD), F32)
    v_out = np.zeros((B, N_KV, T, HD), F32)
    for cid in range(8):
        b, g = divmod(cid, 4)
        out[b] += results[cid]["po"].astype(F32)
        k_out[b, g] = results[cid]["ko"].T.astype(F32)
        v_out[b, g] = results[cid]["vo"].T.astype(F32)
    return out, k_out, v_out


def kernel(x, wq, wk, wv, wo):
    from concourse.bass_utils import run_bass_kernel_spmd

    in_maps = _make_in_maps(x, wq, wk, wv, wo)
    nc = _get_nc()
    res = run_bass_kernel_spmd(nc, in_maps, core_ids=list(range(8)))
    return _gather(res.results)
